# revision 13
# baseline (speedup 1.0000x reference)
"""MoE-routed dynamics MLP on 8 NeuronCores.

Expert-parallel: core p holds expert p's weights. Samples are dispatched
host-side (sort by policy index), each core runs its ~B/P samples through
  concat(latent, action) [C,528] -> H=1024 (relu) -> H=1024 (relu) -> 512
with activations kept transposed ([feature, sample]) so the three GEMMs
chain on the PE without any on-chip transposes:
  h1T = relu(W1.T @ xT + b1),  h2T = relu(W2.T @ h1T + b2),
  outT = W3.T @ h2T + b3.
Matmuls run as float32r (TF32-like: 8-bit exp + 11-bit mantissa; 1 PE
cycle/row for moving dim >=256 vs 4 for plain fp32, fp32 PSUM accumulate).
Weights/inputs are pre-rounded to the f32r grid and pre-tiled host-side
into partition-major 128-row chunks, one DMA per chunk (128 contiguous
2-4KB descriptors each). Layers iterate K-chunk-outer / M-tile-inner so
the PE consumes each weight chunk the moment its DMA lands (just-in-time
streaming); a short bf16 warmup block keeps the PE clock-gate (HAM) warm
while the first chunks arrive. Bias+relu ride the PSUM->SBUF eviction on
ScalarE in fp32.
"""

import numpy as np

P = 8
D_LAT = 512
D_ACT = 16
D_IN = D_LAT + D_ACT  # 528
D_IN_PAD = 640        # 5 x 128
H = 1024
B = 4096

_compiled = {}  # capacity -> nc

# Results of the last run_bass_kernel_spmd call (for external harnesses
# that want exec_time_ns when tracing is enabled via BASS_TRACE).
LAST_RESULT = None


def _round_f32r(a):
    """Round fp32 to the float32r grid (drop low 12 mantissa bits, RNE)."""
    u = np.ascontiguousarray(a).view(np.uint32)
    r = (u + 0x7FF + ((u >> 12) & 1)) & np.uint32(0xFFFFF000)
    return r.view(np.float32)


def _pretile(a):
    """[(k*128), F] row-major -> [128, k*F] partition-major chunks."""
    k = a.shape[0] // 128
    f = a.shape[1]
    return np.ascontiguousarray(
        a[: k * 128].reshape(k, 128, f).transpose(1, 0, 2).reshape(128, k * f)
    )


def _n_slices(C):
    """Split the moving (sample) dim into chunks <=512, balanced so each
    stays >=256 when C >= 512 (float32r full-rate threshold)."""
    k = -(-C // 512)
    base, rem = divmod(C, k)
    sizes = [base + (1 if i < rem else 0) for i in range(k)]
    out = []
    off = 0
    for s in sizes:
        out.append((off, s))
        off += s
    return out


def _build(C):
    import concourse.bacc as bacc
    import concourse.mybir as mybir
    import concourse.tile as tile

    f32 = mybir.dt.float32
    f32r = mybir.dt.float32r
    bf16 = mybir.dt.bfloat16
    AF = mybir.ActivationFunctionType

    nc = bacc.Bacc(None, target_bir_lowering=False)

    x5 = nc.declare_dram_parameter("x5", [128, 5 * C], f32r, isOutput=False)
    w15 = nc.declare_dram_parameter("w15", [128, 5 * H], f32r, isOutput=False)
    bias = nc.declare_dram_parameter("bias", [128, 20], f32, isOutput=False)
    w2 = nc.declare_dram_parameter("w2", [128, 8 * H], f32r, isOutput=False)
    w3 = nc.declare_dram_parameter("w3", [128, 8 * D_LAT], f32r, isOutput=False)
    ot = nc.declare_dram_parameter("ot", [128, 4 * C], f32, isOutput=True)

    m1 = H // 128      # 8 M-tiles for layers 1/2
    m3 = D_LAT // 128  # 4 M-tiles for layer 3
    ns = _n_slices(C)

    with tile.TileContext(nc) as tc:
        with (
            tc.tile_pool(name="xw", bufs=1) as xw,
            tc.tile_pool(name="acts", bufs=1) as acts,
            tc.tile_pool(name="psum", bufs=8, space="PSUM") as psum,
        ):
            # DMA issue order is the stream order: x, W1 chunks, bias, W2
            # chunks, W3 chunks. The Sync sequencer issues these serially
            # (~0.7us each), which keeps later transfers from competing
            # with the ones the PE needs first.
            w1_t = []
            xa_t = xw.tile([128, C], f32r, name="xa_t")
            nc.sync.dma_start(out=xa_t[:], in_=x5[:, :C])
            t = xw.tile([128, H], f32r, name="w1_0")
            nc.sync.dma_start(out=t[:], in_=w15[:, :H])
            w1_t.append(t)
            xb_t = xw.tile([128, C], f32r, name="xb_t")
            nc.sync.dma_start(out=xb_t[:], in_=x5[:, C : 2 * C])
            t = xw.tile([128, H], f32r, name="w1_1")
            nc.sync.dma_start(out=t[:], in_=w15[:, H : 2 * H])
            w1_t.append(t)
            xc_t = xw.tile([128, 3 * C], f32r, name="xc_t")
            nc.sync.dma_start(out=xc_t[:], in_=x5[:, 2 * C :])
            for k in range(2, 5):
                t = xw.tile([128, H], f32r, name=f"w1_{k}")
                nc.sync.dma_start(out=t[:], in_=w15[:, k * H : (k + 1) * H])
                w1_t.append(t)

            def x_at(k, n0, nsz):
                if k == 0:
                    return xa_t[:, n0 : n0 + nsz]
                if k == 1:
                    return xb_t[:, n0 : n0 + nsz]
                return xc_t[:, (k - 2) * C + n0 : (k - 2) * C + n0 + nsz]
            bias_t = xw.tile([128, 20], f32, name="bias_t")
            nc.sync.dma_start(out=bias_t[:], in_=bias[:])
            w2_t = []
            for k in range(8):
                t = xw.tile([128, H], f32r, name=f"w2_{k}")
                nc.sync.dma_start(out=t[:], in_=w2[:, k * H : (k + 1) * H])
                w2_t.append(t)
            w3_t = []
            for k in range(8):
                t = xw.tile([128, D_LAT], f32r, name=f"w3_{k}")
                nc.sync.dma_start(out=t[:], in_=w3[:, k * D_LAT : (k + 1) * D_LAT])
                w3_t.append(t)

            # Warmup: bf16 matmuls with no data dependencies heat the PE
            # clock gate (HAM) while the first chunks stream in.
            wu_s = xw.tile([128, 128], bf16, name="wu_s")
            nc.any.memset(wu_s[:], 0.0)
            wu_m = xw.tile([128, 320], bf16, name="wu_m")
            nc.any.memset(wu_m[:], 0.0)
            wu_p = psum.tile([128, 320], f32, tag="ps", name="wu_p")
            for _ in range(12):
                nc.tensor.matmul(
                    wu_p[:], lhsT=wu_s[:], rhs=wu_m[:], start=True, stop=True
                )

            h1_t = [acts.tile([128, C], f32r, name=f"h1_{m}") for m in range(m1)]
            h2_t = [acts.tile([128, C], f32r, name=f"h2_{m}") for m in range(m1)]
            o_t = [acts.tile([128, C], f32, name=f"o_{m}") for m in range(m3)]

            def layer(w_tiles, rhs_at, out_at, n_m, bias_col, func):
                """One GEMM layer, K-chunk-outer / M-tile-inner per n-pass."""
                n_k = len(w_tiles)
                for n0, nsz in ns:
                    ps = [
                        psum.tile([128, nsz], f32, tag="ps", name=f"ps{m}")
                        for m in range(n_m)
                    ]
                    for k in range(n_k):
                        for m in range(n_m):
                            nc.tensor.matmul(
                                ps[m][:],
                                lhsT=w_tiles[k][:, m * 128 : (m + 1) * 128],
                                rhs=rhs_at(k, n0, nsz),
                                start=(k == 0),
                                stop=(k == n_k - 1),
                            )
                    for m in range(n_m):
                        b = bias_t[:, bias_col + m : bias_col + m + 1]
                        if m % 2 == 0:
                            nc.scalar.activation(
                                out_at(m, n0, nsz), ps[m][:], func, bias=b
                            )
                        elif func == AF.Relu:
                            nc.vector.tensor_scalar(
                                out_at(m, n0, nsz), ps[m][:], b, 0.0,
                                mybir.AluOpType.add, mybir.AluOpType.max,
                            )
                        else:
                            nc.vector.tensor_scalar_add(
                                out_at(m, n0, nsz), ps[m][:], b
                            )

            layer(
                w1_t,
                x_at,
                lambda m, n0, nsz: h1_t[m][:, n0 : n0 + nsz],
                m1, 0, AF.Relu,
            )
            layer(
                w2_t,
                lambda k, n0, nsz: h1_t[k][:, n0 : n0 + nsz],
                lambda m, n0, nsz: h2_t[m][:, n0 : n0 + nsz],
                m1, 8, AF.Relu,
            )
            layer(
                w3_t,
                lambda k, n0, nsz: h2_t[k][:, n0 : n0 + nsz],
                lambda m, n0, nsz: o_t[m][:, n0 : n0 + nsz],
                m3, 16, AF.Identity,
            )

            for n0, nsz in ns:
                for m in range(m3):
                    nc.gpsimd.dma_start(
                        out=ot[:, m * C + n0 : m * C + n0 + nsz],
                        in_=o_t[m][:, n0 : n0 + nsz],
                    )

    nc.compile()
    return nc


def kernel(latents, actions, policy_indices, W1, b1, W2, b2, W3, b3):
    global LAST_RESULT
    from concourse.bass_utils import run_bass_kernel_spmd

    latents = np.ascontiguousarray(np.asarray(latents, dtype=np.float32))
    actions = np.ascontiguousarray(np.asarray(actions, dtype=np.float32))
    idx = np.asarray(policy_indices).astype(np.int64)
    W1 = np.asarray(W1, dtype=np.float32)
    b1 = np.asarray(b1, dtype=np.float32)
    W2 = np.asarray(W2, dtype=np.float32)
    b2 = np.asarray(b2, dtype=np.float32)
    W3 = np.asarray(W3, dtype=np.float32)
    b3 = np.asarray(b3, dtype=np.float32)

    n = latents.shape[0]
    order = np.argsort(idx, kind="stable")
    counts = np.bincount(idx, minlength=P)

    C = max(512, int(-(-counts.max() // 32)) * 32)
    if C not in _compiled:
        _compiled[C] = _build(C)
    nc = _compiled[C]

    x = np.concatenate([latents, actions], axis=1)  # [B, 528]

    in_maps = []
    starts = np.concatenate([[0], np.cumsum(counts)])
    for p in range(P):
        sel = order[starts[p] : starts[p + 1]]
        xp = np.zeros((D_IN_PAD, C), dtype=np.float32)
        xp[:D_IN, : counts[p]] = _round_f32r(np.ascontiguousarray(x[sel].T))
        w1r = np.zeros((D_IN_PAD, H), dtype=np.float32)
        w1r[:D_IN] = _round_f32r(W1[p])
        bp = np.concatenate(
            [
                b1[p].reshape(H // 128, 128).T,
                b2[p].reshape(H // 128, 128).T,
                b3[p].reshape(D_LAT // 128, 128).T,
            ],
            axis=1,
        )
        in_maps.append(
            {
                "x5": _pretile(xp),
                "w15": _pretile(w1r),
                "bias": np.ascontiguousarray(bp),
                "w2": _pretile(_round_f32r(W2[p])),
                "w3": _pretile(_round_f32r(W3[p])),
            }
        )

    res = run_bass_kernel_spmd(nc, in_maps, core_ids=list(range(P)))
    LAST_RESULT = res

    out = np.empty((n, D_LAT), dtype=np.float32)
    for p in range(P):
        sel = order[starts[p] : starts[p + 1]]
        op = res.results[p]["ot"].reshape(128, 4, C).transpose(1, 0, 2).reshape(D_LAT, C)
        out[sel] = op[:, : counts[p]].T
    return out
